# revision 9
# baseline (speedup 1.0000x reference)
"""nn_CombinedLoss Trainium2 kernel (v2).

Computes total/image/station losses for the CombinedLoss module, data-parallel
over the batch dim across 8 NeuronCores.

Per-core device pipeline (B_loc = 4 batches):
  Image loss  mean((P - bilinear_up2x(T))^2):
    - T row-tiles [128,512] -> fused x-upsample (scalar_tensor_tensor) into an
      interleaved bf16 row tile X' (scaled by 4/3 so the scale folds into the
      y-upsample band-matrix weights).
    - y-upsample + subtraction of P as PE matmuls with constant band matrices
      accumulating d = U - P in PSUM (even/odd fine-row quadrants). P enters
      the PE as raw float32 via a float32r bitcast (full rate at ramped
      p-state, exact precision) -- no bf16 cast pass on the Vector engine.
    - ScalarE Square activation with accum_out reduces each PSUM block to
      per-partition partial SSEs.
  Station loss  mean((clipped 3x3 box mean at station - runoff)^2):
    - per station, 3 row-segments of 4 consecutive pixels are gathered with one
      indirect DMA (element-offset table computed on device from positions).
    - masks/counts from positions; masked sum / count; squared diff reduced.
Host sums the per-core partials in f64.

Queue layout: sync issues the big P/T streaming loads; scalar issues the
constant loads (and runs the Square accumulation); gpsimd owns the station
gathers + finalize; vector builds X' and the station index/mask math.
"""

import os
import numpy as np
import ml_dtypes

N_CORES = 8
B_TOT, H, W = 32, 1024, 1024
TH, TW = 512, 512
S = 256
B_LOC = B_TOT // N_CORES  # 4
NT = TH // 128  # 4 target row-tiles per batch
N_IMG_SLOTS = B_LOC * NT  # 16 (one ACT accum slot per target tile)

_CACHE = {}


def _host_consts():
    z = lambda: np.zeros((128, 128), np.float32)
    w0, w1, e0, e1, ni = z(), z(), z(), z(), z()
    for m in range(128):
        w0[m, m] = 0.5625
        if m >= 1:
            w0[m - 1, m] = 0.1875
        w1[m, m] = 0.5625
        if m <= 126:
            w1[m + 1, m] = 0.1875
        ni[m, m] = -1.0
    w0f = w0.copy()
    w0f[0, 0] = 0.75
    w1l = w1.copy()
    w1l[127, 127] = 0.75
    e0[127, 0] = 0.1875
    e1[0, 127] = 0.1875
    wc = np.concatenate([w0, w0f, w1, w1l, e0, e1, ni], axis=1)
    wconst = np.ascontiguousarray(wc.astype(ml_dtypes.bfloat16))
    nif32 = np.ascontiguousarray(ni)

    cconst = np.zeros((128, 8), np.float32)
    p = np.arange(128)
    cconst[:, 0] = (p // 32).astype(np.float32) * float(H * W)  # batch offset
    cconst[:, 1:5] = np.arange(4, dtype=np.float32)[None, :]  # j window offsets
    return wconst, np.ascontiguousarray(cconst), nif32


def _build_nc():
    import concourse.bacc as bacc
    import concourse.bass as bass
    import concourse.mybir as mybir
    from concourse.tile import TileContext
    from contextlib import ExitStack

    f32 = mybir.dt.float32
    f32r = mybir.dt.float32r
    bf16 = mybir.dt.bfloat16
    i32 = mybir.dt.int32
    AL = mybir.AluOpType
    AF = mybir.ActivationFunctionType
    AX = mybir.AxisListType

    # --- build-time knobs (A/B testing) -----------------------------------
    KNI = os.environ.get("KNI", "f32r")  # f32r | bf16: dtype of P into PE
    KMMW = os.environ.get("KMMW", "wide")  # wide | narrow matmul outputs
    _parts = os.environ.get("KPARTS", "both")

    nc = bacc.Bacc(
        "TRN2",
        target_bir_lowering=False,
        debug=False,
        enable_asserts=False,
        num_devices=N_CORES,
    )

    # predictions declared float32r (same bits as f32 host-side) so the PE can
    # consume P at full rate; non-PE readers bitcast back to f32.
    pdt = f32r if KNI == "f32r" else f32
    preds = nc.dram_tensor("predictions", [B_LOC, 1, H, W], pdt, kind="ExternalInput")
    targs = nc.dram_tensor("targets", [B_LOC, 1, TH, TW], f32, kind="ExternalInput")
    pos = nc.dram_tensor("station_positions", [B_LOC, S, 2], i32, kind="ExternalInput")
    runf = nc.dram_tensor("station_runoffs", [B_LOC, S], f32, kind="ExternalInput")
    wc = nc.dram_tensor("wconst", [128, 7 * 128], bf16, kind="ExternalInput")
    cc = nc.dram_tensor("cconst", [128, 8], f32, kind="ExternalInput")
    nif = nc.dram_tensor("nif32", [128, 128], pdt, kind="ExternalInput")
    o_img = nc.dram_tensor("o_img", [128, N_IMG_SLOTS], f32, kind="ExternalOutput")
    o_stn = nc.dram_tensor("o_stn", [128, 1], f32, kind="ExternalOutput")

    with TileContext(nc) as tc:
      with ExitStack() as ctx:
          const_p = ctx.enter_context(tc.tile_pool(name="const", bufs=1))
          stn_p = ctx.enter_context(tc.tile_pool(name="stn", bufs=1))
          tt_p = ctx.enter_context(tc.tile_pool(name="ttp", bufs=6))
          x_p = ctx.enter_context(tc.tile_pool(name="xp", bufs=NT + 2))
          pr_p = ctx.enter_context(tc.tile_pool(name="prp", bufs=6))
          pb_p = ctx.enter_context(tc.tile_pool(name="pbp", bufs=3))
          scr_p = ctx.enter_context(tc.tile_pool(name="scrp", bufs=2))
          ps_p = ctx.enter_context(tc.tile_pool(name="psp", bufs=2, space="PSUM"))

          # ---- constant loads: scalar queue (its own DGE lane, early) ----
          wtile = const_p.tile([128, 7 * 128], bf16)
          nc.scalar.dma_start(out=wtile[:], in_=wc[:])
          names = ["w0", "w0f", "w1", "w1l", "e0", "e1", "ni"]
          Wm = {k: wtile[:, 128 * i : 128 * (i + 1)] for i, k in enumerate(names)}
          ctile = const_p.tile([128, 8], f32)
          nc.scalar.dma_start(out=ctile[:], in_=cc[:])
          niftile = const_p.tile([128, 128], pdt)
          nc.scalar.dma_start(out=niftile[:], in_=nif[:])
          boff = ctile[:, 0:1]
          joff = ctile[:, 1:5]

          # station inputs: gpsimd SWDGE lane
          if _parts in ("both", "stn"):
            pos_t = stn_p.tile([128, 16], i32)
            nc.gpsimd.dma_start(
                out=pos_t[:], in_=pos[:].rearrange("b (s2 s8) k -> (b s2) (s8 k)", s8=8)
            )
            run_t = stn_p.tile([128, 8], f32)
            nc.gpsimd.dma_start(
                out=run_t[:], in_=runf[:].rearrange("b (s2 s8) -> (b s2) s8", s8=8)
            )

          oimg_t = stn_p.tile([128, N_IMG_SLOTS], f32)
          ostn_t = stn_p.tile([128, 1], f32)

          third = 1.0 / 3.0
          xts_by_b = {}
          pfulls_by_b = {}

          def emit_ploads(b):
              pfulls = []
              for t in range(NT):
                  # T tile first (small; X' build gates the first matmuls)
                  ttile = tt_p.tile([128, TW], f32)
                  nc.sync.dma_start(
                      out=ttile[:], in_=targs[b, 0, 128 * t : 128 * (t + 1), :]
                  )
                  # one contiguous 1 MB load: partition p holds fine rows
                  # (256t+2p, 256t+2p+1) -> [even-row cols | odd-row cols]
                  pfull = pr_p.tile([128, 2 * W], pdt)
                  r0 = 256 * t
                  nc.sync.dma_start(
                      out=pfull[:],
                      in_=preds[b, 0, r0 : r0 + 256, :].rearrange(
                          "(p two) w -> p (two w)", two=2
                      ),
                  )
                  pfulls.append((ttile, pfull))
              pfulls_by_b[b] = pfulls

          def emit_xphase(b):
              xts = []
              for t in range(NT):
                  ttile = pfulls_by_b[b][t][0]
                  xt = x_p.tile([128, 2 * TW], bf16)
                  # even fine cols: X'[2i] = T[i] + T[i-1]/3   (i = 1..511)
                  nc.vector.scalar_tensor_tensor(
                      out=xt[:, 2 : 2 * TW : 2],
                      in0=ttile[:, 0 : TW - 1],
                      scalar=third,
                      in1=ttile[:, 1:TW],
                      op0=AL.mult,
                      op1=AL.add,
                  )
                  nc.vector.tensor_scalar(xt[:, 0:1], ttile[:, 0:1], 4.0 / 3.0, None, AL.mult)
                  # odd fine cols: X'[2i+1] = T[i] + T[i+1]/3  (i = 0..510)
                  nc.vector.scalar_tensor_tensor(
                      out=xt[:, 1 : 2 * TW - 1 : 2],
                      in0=ttile[:, 1:TW],
                      scalar=third,
                      in1=ttile[:, 0 : TW - 1],
                      op0=AL.mult,
                      op1=AL.add,
                  )
                  nc.vector.tensor_scalar(
                      xt[:, 2 * TW - 1 : 2 * TW], ttile[:, TW - 1 : TW], 4.0 / 3.0, None, AL.mult
                  )
                  xts.append(xt)
              xts_by_b[b] = xts

          def emit_station_idx():
            # vector-side index/mask math for the station gathers
            posf = stn_p.tile([128, 16], f32)
            nc.vector.tensor_copy(posf[:], pos_t[:])
            px = posf[:, 0:16:2]
            py = posf[:, 1:16:2]

            xs0 = stn_p.tile([128, 8], f32)
            nc.vector.tensor_scalar(xs0[:], px, -1.0, 0.0, AL.add, AL.max)
            nc.vector.tensor_scalar(xs0[:], xs0[:], float(W - 4), None, AL.min)

            # indices: fidx[p, s*3 + dyi] = clip(py+dy)*W + xs0  (+ batch offset)
            fidx = stn_p.tile([128, 24], f32)
            yc = stn_p.tile([128, 8], f32)
            for dyi, dy in enumerate((-1.0, 0.0, 1.0)):
                nc.vector.tensor_scalar(yc[:], py, dy, 0.0, AL.add, AL.max)
                nc.vector.tensor_scalar(yc[:], yc[:], float(H - 1), None, AL.min)
                nc.vector.scalar_tensor_tensor(
                    out=fidx[:, dyi:24:3],
                    in0=yc[:],
                    scalar=float(W),
                    in1=xs0[:],
                    op0=AL.mult,
                    op1=AL.add,
                )
            nc.vector.tensor_scalar(fidx[:], fidx[:], boff, None, AL.add)
            idx_t = stn_p.tile([128, 24], i32)
            nc.vector.tensor_copy(idx_t[:], fidx[:])
            return px, py, xs0, idx_t

          def emit_station_gathers(idx_t):
            # HW indirect DMA honors ONE index per partition per instruction
            # (verified: extra free-dim indices are ignored; the transfer is
            # out-free-size contiguous elements from the first index). So:
            # 24 gathers, one per (station-slot s, dy) pair. gpsimd-only.
            g_t = stn_p.tile([128, 96], f32)
            pred_flat = preds[:].bitcast(f32).rearrange("b c h w -> (b c h) w")
            for k in range(24):
                nc.gpsimd.indirect_dma_start(
                    out=g_t[:, 4 * k : 4 * k + 4],
                    out_offset=None,
                    in_=pred_flat,
                    in_offset=bass.IndirectOffsetOnAxis(ap=idx_t[:, k : k + 1], axis=1),
                )
            return g_t

          def emit_station_masks(px, py, xs0):
            tx = stn_p.tile([128, 8], f32)
            nc.vector.tensor_tensor(tx[:], xs0[:], px, AL.subtract)
            wx = stn_p.tile([128, 32], f32)
            nc.vector.tensor_tensor(
                wx[:].rearrange("p (s j) -> p s j", j=4),
                tx[:].unsqueeze(2).broadcast_to([128, 8, 4]),
                joff.unsqueeze(1).broadcast_to([128, 8, 4]),
                AL.add,
            )
            nc.vector.tensor_tensor(wx[:], wx[:], wx[:], AL.mult)
            nc.vector.tensor_scalar(wx[:], wx[:], 1.5, None, AL.is_le)

            vy3 = stn_p.tile([128, 24], f32)
            nc.vector.tensor_scalar(vy3[:, 0:24:3], py, 1.0, None, AL.is_ge)
            nc.vector.tensor_scalar(vy3[:, 1:24:3], py, 0.0, None, AL.is_ge)
            nc.vector.tensor_scalar(vy3[:, 2:24:3], py, float(H - 2), None, AL.is_le)

            mask = stn_p.tile([128, 96], f32)
            nc.vector.tensor_tensor(
                mask[:].rearrange("p (s d j) -> p s d j", d=3, j=4),
                vy3[:].rearrange("p (s d) -> p s d", d=3).unsqueeze(3).broadcast_to([128, 8, 3, 4]),
                wx[:].rearrange("p (s j) -> p s j", j=4).unsqueeze(2).broadcast_to([128, 8, 3, 4]),
                AL.mult,
            )

            cy = stn_p.tile([128, 8], f32)
            nc.vector.tensor_reduce(
                cy[:], vy3[:].rearrange("p (s d) -> p s d", d=3), AX.X, AL.add
            )
            cx = stn_p.tile([128, 8], f32)
            nc.vector.tensor_reduce(
                cx[:], wx[:].rearrange("p (s j) -> p s j", j=4), AX.X, AL.add
            )
            cnt = stn_p.tile([128, 8], f32)
            nc.vector.tensor_tensor(cnt[:], cy[:], cx[:], AL.mult)
            rcnt = stn_p.tile([128, 8], f32)
            nc.vector.reciprocal(rcnt[:], cnt[:])
            return mask, rcnt

          def emit_stn_finalize(g_t, mask, rcnt):
            # all on GPSIMD: keeps gather-dependent work off the (busy, in-order)
            # Vector queue so a scheduler mis-ordering can't head-of-line block it
            gm = stn_p.tile([128, 96], f32)
            nc.gpsimd.tensor_tensor(gm[:], g_t[:], mask[:], AL.mult)
            # group-of-12 sum via tree adds (gpsimd has no free-axis reduce)
            gv = lambda a, b: gm[:].rearrange("p (s e) -> p s e", e=12)[:, :, a:b]
            t6 = stn_p.tile([128, 48], f32)
            t6v = t6[:].rearrange("p (s e) -> p s e", e=6)
            nc.gpsimd.tensor_tensor(t6v, gv(0, 6), gv(6, 12), AL.add)
            t3 = stn_p.tile([128, 24], f32)
            t3v = t3[:].rearrange("p (s e) -> p s e", e=3)
            nc.gpsimd.tensor_tensor(t3v, t6v[:, :, 0:3], t6v[:, :, 3:6], AL.add)
            bsum = stn_p.tile([128, 8], f32)
            nc.gpsimd.tensor_tensor(
                bsum[:], t3v[:, :, 0], t3v[:, :, 1], AL.add
            )
            nc.gpsimd.tensor_tensor(bsum[:], bsum[:], t3v[:, :, 2], AL.add)
            d_t = stn_p.tile([128, 8], f32)
            nc.gpsimd.tensor_tensor(d_t[:], bsum[:], rcnt[:], AL.mult)
            nc.gpsimd.tensor_tensor(d_t[:], d_t[:], run_t[:], AL.subtract)
            scr8 = stn_p.tile([128, 8], f32)
            nc.gpsimd.tensor_tensor(scr8[:], d_t[:], d_t[:], AL.mult)
            s4 = stn_p.tile([128, 4], f32)
            nc.gpsimd.tensor_tensor(s4[:], scr8[:, 0:4], scr8[:, 4:8], AL.add)
            s2 = stn_p.tile([128, 2], f32)
            nc.gpsimd.tensor_tensor(s2[:], s4[:, 0:2], s4[:, 2:4], AL.add)
            nc.gpsimd.tensor_tensor(ostn_t[:], s2[:, 0:1], s2[:, 1:2], AL.add)
            nc.sync.dma_start(out=o_stn[:], in_=ostn_t[:])

          def emit_img_tile(b, t):
              xts = xts_by_b[b]
              pfull = pfulls_by_b[b][t][1]
              xt = xts[t]
              # one 4-bank PSUM tile per target tile: [r0h0 | r0h1 | r1h0 | r1h1]
              ps = ps_p.tile([128, 2 * W], f32, space="PSUM")
              w0k = Wm["w0f"] if t == 0 else Wm["w0"]
              w1k = Wm["w1l"] if t == NT - 1 else Wm["w1"]

              if KNI == "f32r":
                  prhs = pfull[:]
                  nilhs = niftile[:]
              else:
                  pbf = pb_p.tile([128, 2 * W], bf16)
                  nc.vector.tensor_copy(pbf[:], pfull[:])
                  prhs = pbf[:]
                  nilhs = Wm["ni"]

              if KMMW == "wide":
                  # 2-bank (1024-col) upsample matmuls, 4-bank (2048-col) subtract
                  nc.tensor.matmul(
                      out=ps[:, 0:1024], lhsT=w0k, rhs=xt[:, 0:1024],
                      start=True, stop=False,
                  )
                  nc.tensor.matmul(
                      out=ps[:, 1024:2048], lhsT=w1k, rhs=xt[:, 0:1024],
                      start=True, stop=False,
                  )
                  if t > 0:
                      nc.tensor.matmul(
                          out=ps[:, 0:1024], lhsT=Wm["e0"], rhs=xts[t - 1][:, 0:1024],
                          start=False, stop=False,
                      )
                  if t < NT - 1:
                      nc.tensor.matmul(
                          out=ps[:, 1024:2048], lhsT=Wm["e1"], rhs=xts[t + 1][:, 0:1024],
                          start=False, stop=False,
                      )
                  nc.tensor.matmul(
                      out=ps[:, 0:2048], lhsT=nilhs, rhs=prhs,
                      start=False, stop=True, skip_group_check=True,
                  )
              else:
                  bank = lambda q: slice(512 * q, 512 * (q + 1))
                  col = lambda h: slice(512 * h, 512 * (h + 1))
                  for h in range(2):
                      nc.tensor.matmul(
                          out=ps[:, bank(h)], lhsT=w0k, rhs=xt[:, col(h)],
                          start=True, stop=False,
                      )
                  for h in range(2):
                      nc.tensor.matmul(
                          out=ps[:, bank(2 + h)], lhsT=w1k, rhs=xt[:, col(h)],
                          start=True, stop=False,
                      )
                  if t > 0:
                      for h in range(2):
                          nc.tensor.matmul(
                              out=ps[:, bank(h)], lhsT=Wm["e0"], rhs=xts[t - 1][:, col(h)],
                              start=False, stop=False,
                          )
                  if t < NT - 1:
                      for h in range(2):
                          nc.tensor.matmul(
                              out=ps[:, bank(2 + h)], lhsT=Wm["e1"], rhs=xts[t + 1][:, col(h)],
                              start=False, stop=False,
                          )
                  for q in range(4):
                      nc.tensor.matmul(
                          out=ps[:, bank(q)], lhsT=nilhs,
                          rhs=prhs[:, bank(q)],
                          start=False, stop=True, skip_group_check=True,
                      )

              slot = b * NT + t
              scr0 = scr_p.tile([128, 2 * W], bf16)
              nc.scalar.activation(
                  out=scr0[:], in_=ps[:], func=AF.Square,
                  accum_out=oimg_t[:, slot : slot + 1],
              )

          # ------------------- emission schedule -------------------
          if _parts == "stn":
              px, py, xs0, idx_t = emit_station_idx()
              g_t = emit_station_gathers(idx_t)
              mask, rcnt = emit_station_masks(px, py, xs0)
              emit_stn_finalize(g_t, mask, rcnt)
          else:
              emit_ploads(0)
              emit_xphase(0)
              if _parts == "both":
                  px, py, xs0, idx_t = emit_station_idx()
                  g_t = emit_station_gathers(idx_t)
              emit_ploads(1)
              for t in range(NT):
                  emit_img_tile(0, t)
              emit_xphase(1)
              if _parts == "both":
                  mask, rcnt = emit_station_masks(px, py, xs0)
              emit_ploads(2)
              for t in range(NT):
                  emit_img_tile(1, t)
              emit_xphase(2)
              if _parts == "both":
                  emit_stn_finalize(g_t, mask, rcnt)
              emit_ploads(3)
              for t in range(NT):
                  emit_img_tile(2, t)
              emit_xphase(3)
              for t in range(NT):
                  emit_img_tile(3, t)
              nc.sync.dma_start(out=o_img[:], in_=oimg_t[:])

    nc.compile()
    return nc


def _get_nc():
    if "nc" not in _CACHE:
        _CACHE["nc"] = _build_nc()
    return _CACHE["nc"]


def _in_maps(inputs):
    wconst, cconst, nif32 = _host_consts()
    preds = np.ascontiguousarray(np.asarray(inputs["predictions"], dtype=np.float32))
    targs = np.ascontiguousarray(np.asarray(inputs["targets"], dtype=np.float32))
    pos = np.ascontiguousarray(np.asarray(inputs["station_positions"], dtype=np.int32))
    runf = np.ascontiguousarray(np.asarray(inputs["station_runoffs"], dtype=np.float32))
    maps = []
    for c in range(N_CORES):
        sl = slice(c * B_LOC, (c + 1) * B_LOC)
        maps.append(
            {
                "predictions": np.ascontiguousarray(preds[sl]),
                "targets": np.ascontiguousarray(targs[sl]),
                "station_positions": np.ascontiguousarray(pos[sl]),
                "station_runoffs": np.ascontiguousarray(runf[sl]),
                "wconst": wconst,
                "cconst": cconst,
                "nif32": nif32,
            }
        )
    return maps


def _postprocess(results):
    img_sse = 0.0
    stn_sse = 0.0
    for r in results:
        img_sse += float(r["o_img"].astype(np.float64).sum())
        stn_sse += float(r["o_stn"].astype(np.float64).sum())
    img_loss = img_sse / float(B_TOT * H * W)
    stn_loss = stn_sse / float(B_TOT * S)
    total = 1.0 * img_loss + 0.5 * stn_loss
    return (
        np.float32(total),
        np.float32(img_loss),
        np.float32(stn_loss),
    )


def run(inputs, **run_kwargs):
    """Run the kernel; returns (BassKernelResults, (total, img, stn))."""
    from concourse.bass_utils import run_bass_kernel_spmd

    nc = _get_nc()
    res = run_bass_kernel_spmd(
        nc, _in_maps(inputs), core_ids=list(range(N_CORES)), **run_kwargs
    )
    return res, _postprocess(res.results)


def kernel(**inputs):
    _, out = run(inputs)
    return out


# revision 14
# speedup vs baseline: 1.1133x; 1.1133x over previous
"""nn_CombinedLoss Trainium2 kernel (v2).

Computes total/image/station losses for the CombinedLoss module, data-parallel
over the batch dim across 8 NeuronCores.

Per-core device pipeline (B_loc = 4 batches):
  Image loss  mean((P - bilinear_up2x(T))^2):
    - T row-tiles [128,512] -> fused x-upsample (scalar_tensor_tensor) into an
      interleaved bf16 row tile X' (scaled by 4/3 so the scale folds into the
      y-upsample band-matrix weights).
    - y-upsample + subtraction of P as PE matmuls with constant band matrices
      accumulating d = U - P in PSUM (even/odd fine-row quadrants). P enters
      the PE as raw float32 via a float32r bitcast (full rate at ramped
      p-state, exact precision) -- no bf16 cast pass on the Vector engine.
    - ScalarE Square activation with accum_out reduces each PSUM block to
      per-partition partial SSEs.
  Station loss  mean((clipped 3x3 box mean at station - runoff)^2):
    - per station, 3 row-segments of 4 consecutive pixels are gathered with one
      indirect DMA (element-offset table computed on device from positions).
    - masks/counts from positions; masked sum / count; squared diff reduced.
Host sums the per-core partials in f64.

Queue layout: sync issues the big P/T streaming loads; scalar issues the
constant loads (and runs the Square accumulation); gpsimd owns the station
gathers + finalize; vector builds X' and the station index/mask math.
"""

import os
import numpy as np
import ml_dtypes

N_CORES = 8
B_TOT, H, W = 32, 1024, 1024
TH, TW = 512, 512
S = 256
B_LOC = B_TOT // N_CORES  # 4
NT = TH // 128  # 4 target row-tiles per batch
N_IMG_SLOTS = B_LOC * NT  # 16 (one ACT accum slot per target tile)

_CACHE = {}


def _host_consts():
    z = lambda: np.zeros((128, 128), np.float32)
    w0, w1, e0, e1, ni = z(), z(), z(), z(), z()
    for m in range(128):
        w0[m, m] = 0.5625
        if m >= 1:
            w0[m - 1, m] = 0.1875
        w1[m, m] = 0.5625
        if m <= 126:
            w1[m + 1, m] = 0.1875
        ni[m, m] = -1.0
    w0f = w0.copy()
    w0f[0, 0] = 0.75
    w1l = w1.copy()
    w1l[127, 127] = 0.75
    e0[127, 0] = 0.1875
    e1[0, 127] = 0.1875
    wc = np.concatenate([w0, w0f, w1, w1l, e0, e1, ni], axis=1)
    wconst = np.ascontiguousarray(wc.astype(ml_dtypes.bfloat16))
    nif32 = np.ascontiguousarray(ni)

    cconst = np.zeros((128, 8), np.float32)
    p = np.arange(128)
    cconst[:, 0] = (p // 32).astype(np.float32) * float(H * W)  # batch offset
    cconst[:, 1:5] = np.arange(4, dtype=np.float32)[None, :]  # j window offsets
    return wconst, np.ascontiguousarray(cconst), nif32


def _build_nc():
    import concourse.bacc as bacc
    import concourse.bass as bass
    import concourse.mybir as mybir
    from concourse.tile import TileContext
    from contextlib import ExitStack

    f32 = mybir.dt.float32
    f32r = mybir.dt.float32r
    bf16 = mybir.dt.bfloat16
    i32 = mybir.dt.int32
    AL = mybir.AluOpType
    AF = mybir.ActivationFunctionType
    AX = mybir.AxisListType

    # --- build-time knobs (A/B testing) -----------------------------------
    KNI = os.environ.get("KNI", "f32r")  # f32r | bf16: dtype of P into PE
    KMMW = os.environ.get("KMMW", "narrow")  # narrow (512-col, ISA max) | wide
    _parts = os.environ.get("KPARTS", "both")

    nc = bacc.Bacc(
        "TRN2",
        target_bir_lowering=False,
        debug=False,
        enable_asserts=False,
        num_devices=N_CORES,
    )

    # predictions declared float32r (same bits as f32 host-side) so the PE can
    # consume P at full rate; non-PE readers bitcast back to f32.
    pdt = f32r if KNI == "f32r" else f32
    preds = nc.dram_tensor("predictions", [B_LOC, 1, H, W], pdt, kind="ExternalInput")
    targs = nc.dram_tensor("targets", [B_LOC, 1, TH, TW], f32, kind="ExternalInput")
    pos = nc.dram_tensor("station_positions", [B_LOC, S, 2], i32, kind="ExternalInput")
    runf = nc.dram_tensor("station_runoffs", [B_LOC, S], f32, kind="ExternalInput")
    wc = nc.dram_tensor("wconst", [128, 7 * 128], bf16, kind="ExternalInput")
    cc = nc.dram_tensor("cconst", [128, 8], f32, kind="ExternalInput")
    nif = nc.dram_tensor("nif32", [128, 128], pdt, kind="ExternalInput")
    o_img = nc.dram_tensor("o_img", [128, N_IMG_SLOTS], f32, kind="ExternalOutput")
    o_stn = nc.dram_tensor("o_stn", [128, 1], f32, kind="ExternalOutput")

    with TileContext(nc) as tc:
      with ExitStack() as ctx:
          const_p = ctx.enter_context(tc.tile_pool(name="const", bufs=1))
          stn_p = ctx.enter_context(tc.tile_pool(name="stn", bufs=1))
          tt_p = ctx.enter_context(tc.tile_pool(name="ttp", bufs=6))
          x_p = ctx.enter_context(tc.tile_pool(name="xp", bufs=NT + 2))
          pr_p = ctx.enter_context(tc.tile_pool(name="prp", bufs=6))
          pb_p = ctx.enter_context(tc.tile_pool(name="pbp", bufs=3))
          scr_p = ctx.enter_context(tc.tile_pool(name="scrp", bufs=2))
          ps_p = ctx.enter_context(tc.tile_pool(name="psp", bufs=2, space="PSUM"))

          # ---- constant loads: scalar queue (its own DGE lane, early) ----
          wtile = const_p.tile([128, 7 * 128], bf16)
          nc.scalar.dma_start(out=wtile[:], in_=wc[:])
          names = ["w0", "w0f", "w1", "w1l", "e0", "e1", "ni"]
          Wm = {k: wtile[:, 128 * i : 128 * (i + 1)] for i, k in enumerate(names)}
          ctile = const_p.tile([128, 8], f32)
          nc.scalar.dma_start(out=ctile[:], in_=cc[:])
          niftile = const_p.tile([128, 128], pdt)
          nc.scalar.dma_start(out=niftile[:], in_=nif[:])
          boff = ctile[:, 0:1]
          joff = ctile[:, 1:5]

          # station inputs: gpsimd SWDGE lane
          if _parts in ("both", "stn"):
            pos_t = stn_p.tile([128, 16], i32)
            nc.gpsimd.dma_start(
                out=pos_t[:], in_=pos[:].rearrange("b (s2 s8) k -> (b s2) (s8 k)", s8=8)
            )
            run_t = stn_p.tile([128, 8], f32)
            nc.gpsimd.dma_start(
                out=run_t[:], in_=runf[:].rearrange("b (s2 s8) -> (b s2) s8", s8=8)
            )

          oimg_t = stn_p.tile([128, N_IMG_SLOTS], f32)
          ostn_t = stn_p.tile([128, 1], f32)

          third = 1.0 / 3.0
          xts_by_b = {}
          pfulls_by_b = {}

          def emit_ploads(b):
              pfulls = []
              for t in range(NT):
                  # T tile first (small; X' build gates the first matmuls)
                  ttile = tt_p.tile([128, TW], f32)
                  nc.sync.dma_start(
                      out=ttile[:], in_=targs[b, 0, 128 * t : 128 * (t + 1), :]
                  )
                  # one contiguous 1 MB load: partition p holds fine rows
                  # (256t+2p, 256t+2p+1) -> [even-row cols | odd-row cols]
                  pfull = pr_p.tile([128, 2 * W], pdt)
                  r0 = 256 * t
                  nc.sync.dma_start(
                      out=pfull[:],
                      in_=preds[b, 0, r0 : r0 + 256, :].rearrange(
                          "(p two) w -> p (two w)", two=2
                      ),
                  )
                  pfulls.append((ttile, pfull))
              pfulls_by_b[b] = pfulls

          def emit_xphase(b):
              xts = []
              for t in range(NT):
                  ttile = pfulls_by_b[b][t][0]
                  xt = x_p.tile([128, 2 * TW], bf16)
                  # even fine cols: X'[2i] = T[i] + T[i-1]/3   (i = 1..511)
                  nc.vector.scalar_tensor_tensor(
                      out=xt[:, 2 : 2 * TW : 2],
                      in0=ttile[:, 0 : TW - 1],
                      scalar=third,
                      in1=ttile[:, 1:TW],
                      op0=AL.mult,
                      op1=AL.add,
                  )
                  nc.vector.tensor_scalar(xt[:, 0:1], ttile[:, 0:1], 4.0 / 3.0, None, AL.mult)
                  # odd fine cols: X'[2i+1] = T[i] + T[i+1]/3  (i = 0..510)
                  nc.vector.scalar_tensor_tensor(
                      out=xt[:, 1 : 2 * TW - 1 : 2],
                      in0=ttile[:, 1:TW],
                      scalar=third,
                      in1=ttile[:, 0 : TW - 1],
                      op0=AL.mult,
                      op1=AL.add,
                  )
                  nc.vector.tensor_scalar(
                      xt[:, 2 * TW - 1 : 2 * TW], ttile[:, TW - 1 : TW], 4.0 / 3.0, None, AL.mult
                  )
                  xts.append(xt)
              xts_by_b[b] = xts

          def emit_station_idx():
            # vector-side index/mask math for the station gathers
            posf = stn_p.tile([128, 16], f32)
            nc.vector.tensor_copy(posf[:], pos_t[:])
            px = posf[:, 0:16:2]
            py = posf[:, 1:16:2]

            xs0 = stn_p.tile([128, 8], f32)
            nc.vector.tensor_scalar(xs0[:], px, -1.0, 0.0, AL.add, AL.max)
            nc.vector.tensor_scalar(xs0[:], xs0[:], float(W - 4), None, AL.min)

            # indices: fidx[p, s*3 + dyi] = clip(py+dy)*W + xs0  (+ batch offset)
            fidx = stn_p.tile([128, 24], f32)
            yc = stn_p.tile([128, 8], f32)
            for dyi, dy in enumerate((-1.0, 0.0, 1.0)):
                nc.vector.tensor_scalar(yc[:], py, dy, 0.0, AL.add, AL.max)
                nc.vector.tensor_scalar(yc[:], yc[:], float(H - 1), None, AL.min)
                nc.vector.scalar_tensor_tensor(
                    out=fidx[:, dyi:24:3],
                    in0=yc[:],
                    scalar=float(W),
                    in1=xs0[:],
                    op0=AL.mult,
                    op1=AL.add,
                )
            nc.vector.tensor_scalar(fidx[:], fidx[:], boff, None, AL.add)
            idx_t = stn_p.tile([128, 24], i32)
            nc.vector.tensor_copy(idx_t[:], fidx[:])
            return px, py, xs0, idx_t

          def emit_station_gathers(idx_t):
            # HW indirect DMA honors ONE index per partition per instruction
            # (verified: extra free-dim indices are ignored; the transfer is
            # out-free-size contiguous elements from the first index). So:
            # 24 gathers, one per (station-slot s, dy) pair. gpsimd-only.
            # Three per-dy destination tiles, gathers round-robin across them:
            # consecutive gathers hit different tiles so the WAW completion
            # chain (desc-gen + wire + sem ~2us) never stalls the queue.
            g_dy = [stn_p.tile([128, 32], f32, name=f"g_dy{i}") for i in range(3)]
            pred_flat = preds[:].bitcast(f32).rearrange("b c h w -> (b c h) w")
            for s in range(8):
                for dyi in range(3):
                    nc.gpsimd.indirect_dma_start(
                        out=g_dy[dyi][:, 4 * s : 4 * s + 4],
                        out_offset=None,
                        in_=pred_flat,
                        in_offset=bass.IndirectOffsetOnAxis(
                            ap=idx_t[:, 3 * s + dyi : 3 * s + dyi + 1], axis=1
                        ),
                    )
            return g_dy

          def emit_station_masks(px, py, xs0):
            tx = stn_p.tile([128, 8], f32)
            nc.vector.tensor_tensor(tx[:], xs0[:], px, AL.subtract)
            # wx[:, 4s+j] = |xs0_s + j - px_s| <= 1  -- x-window mask, (s,j)
            wx = stn_p.tile([128, 32], f32)
            nc.vector.tensor_tensor(
                wx[:].rearrange("p (s j) -> p s j", j=4),
                tx[:].unsqueeze(2).broadcast_to([128, 8, 4]),
                joff.unsqueeze(1).broadcast_to([128, 8, 4]),
                AL.add,
            )
            nc.vector.tensor_tensor(wx[:], wx[:], wx[:], AL.mult)
            nc.vector.tensor_scalar(wx[:], wx[:], 1.5, None, AL.is_le)

            # per-dy row-validity (dy=0 is always valid: py in [0, H))
            vy_m1 = stn_p.tile([128, 8], f32)
            nc.vector.tensor_scalar(vy_m1[:], py, 1.0, None, AL.is_ge)
            vy_p1 = stn_p.tile([128, 8], f32)
            nc.vector.tensor_scalar(vy_p1[:], py, float(H - 2), None, AL.is_le)

            # masked per-dy (s,j) masks: mask_dy = wx * vy_dy
            mask_m1 = stn_p.tile([128, 32], f32)
            nc.vector.tensor_tensor(
                mask_m1[:].rearrange("p (s j) -> p s j", j=4),
                wx[:].rearrange("p (s j) -> p s j", j=4),
                vy_m1[:].unsqueeze(2).broadcast_to([128, 8, 4]),
                AL.mult,
            )
            mask_p1 = stn_p.tile([128, 32], f32)
            nc.vector.tensor_tensor(
                mask_p1[:].rearrange("p (s j) -> p s j", j=4),
                wx[:].rearrange("p (s j) -> p s j", j=4),
                vy_p1[:].unsqueeze(2).broadcast_to([128, 8, 4]),
                AL.mult,
            )

            cy = stn_p.tile([128, 8], f32)
            nc.vector.tensor_tensor(cy[:], vy_m1[:], vy_p1[:], AL.add)
            nc.vector.tensor_scalar(cy[:], cy[:], 1.0, None, AL.add)
            cx = stn_p.tile([128, 8], f32)
            nc.vector.tensor_reduce(
                cx[:], wx[:].rearrange("p (s j) -> p s j", j=4), AX.X, AL.add
            )
            cnt = stn_p.tile([128, 8], f32)
            nc.vector.tensor_tensor(cnt[:], cy[:], cx[:], AL.mult)
            rcnt = stn_p.tile([128, 8], f32)
            nc.vector.reciprocal(rcnt[:], cnt[:])
            return (mask_m1, wx, mask_p1), rcnt

          def emit_stn_finalize(g_dy, masks, rcnt):
            # all on GPSIMD: keeps gather-dependent work off the (busy, in-order)
            # Vector queue so a scheduler mis-ordering can't head-of-line block it
            acc = stn_p.tile([128, 32], f32)
            gm1 = stn_p.tile([128, 32], f32)
            nc.gpsimd.tensor_tensor(acc[:], g_dy[0][:], masks[0][:], AL.mult)
            nc.gpsimd.tensor_tensor(gm1[:], g_dy[1][:], masks[1][:], AL.mult)
            nc.gpsimd.tensor_tensor(acc[:], acc[:], gm1[:], AL.add)
            nc.gpsimd.tensor_tensor(gm1[:], g_dy[2][:], masks[2][:], AL.mult)
            nc.gpsimd.tensor_tensor(acc[:], acc[:], gm1[:], AL.add)
            # j-tree: (s,4) -> (s,2) -> (s,1)
            accv = acc[:].rearrange("p (s j) -> p s j", j=4)
            h2 = stn_p.tile([128, 16], f32)
            h2v = h2[:].rearrange("p (s j) -> p s j", j=2)
            nc.gpsimd.tensor_tensor(h2v, accv[:, :, 0:2], accv[:, :, 2:4], AL.add)
            bsum = stn_p.tile([128, 8], f32)
            nc.gpsimd.tensor_tensor(bsum[:], h2v[:, :, 0], h2v[:, :, 1], AL.add)
            d_t = stn_p.tile([128, 8], f32)
            nc.gpsimd.tensor_tensor(d_t[:], bsum[:], rcnt[:], AL.mult)
            nc.gpsimd.tensor_tensor(d_t[:], d_t[:], run_t[:], AL.subtract)
            scr8 = stn_p.tile([128, 8], f32)
            nc.gpsimd.tensor_tensor(scr8[:], d_t[:], d_t[:], AL.mult)
            s4 = stn_p.tile([128, 4], f32)
            nc.gpsimd.tensor_tensor(s4[:], scr8[:, 0:4], scr8[:, 4:8], AL.add)
            s2 = stn_p.tile([128, 2], f32)
            nc.gpsimd.tensor_tensor(s2[:], s4[:, 0:2], s4[:, 2:4], AL.add)
            nc.gpsimd.tensor_tensor(ostn_t[:], s2[:, 0:1], s2[:, 1:2], AL.add)
            # o_stn goes out on gpsimd's own SWDGE lane: it must NOT sit in the
            # sync queue mid-stream, where it would head-of-line block the
            # remaining P/T load issues behind the whole station dep chain.
            nc.gpsimd.dma_start(out=o_stn[:], in_=ostn_t[:])

          def emit_img_tile(b, t):
              xts = xts_by_b[b]
              pfull = pfulls_by_b[b][t][1]
              xt = xts[t]
              # one 4-bank PSUM tile per target tile: [r0h0 | r0h1 | r1h0 | r1h1]
              ps = ps_p.tile([128, 2 * W], f32, space="PSUM")
              w0k = Wm["w0f"] if t == 0 else Wm["w0"]
              w1k = Wm["w1l"] if t == NT - 1 else Wm["w1"]

              if KNI == "f32r":
                  prhs = pfull[:]
                  nilhs = niftile[:]
              else:
                  pbf = pb_p.tile([128, 2 * W], bf16)
                  nc.vector.tensor_copy(pbf[:], pfull[:])
                  prhs = pbf[:]
                  nilhs = Wm["ni"]

              if KMMW == "wide":
                  # 2-bank (1024-col) upsample matmuls, 4-bank (2048-col) subtract
                  nc.tensor.matmul(
                      out=ps[:, 0:1024], lhsT=w0k, rhs=xt[:, 0:1024],
                      start=True, stop=False,
                  )
                  nc.tensor.matmul(
                      out=ps[:, 1024:2048], lhsT=w1k, rhs=xt[:, 0:1024],
                      start=True, stop=False,
                  )
                  if t > 0:
                      nc.tensor.matmul(
                          out=ps[:, 0:1024], lhsT=Wm["e0"], rhs=xts[t - 1][:, 0:1024],
                          start=False, stop=False,
                      )
                  if t < NT - 1:
                      nc.tensor.matmul(
                          out=ps[:, 1024:2048], lhsT=Wm["e1"], rhs=xts[t + 1][:, 0:1024],
                          start=False, stop=False,
                      )
                  nc.tensor.matmul(
                      out=ps[:, 0:2048], lhsT=nilhs, rhs=prhs,
                      start=False, stop=True, skip_group_check=True,
                  )
              else:
                  bank = lambda q: slice(512 * q, 512 * (q + 1))
                  col = lambda h: slice(512 * h, 512 * (h + 1))
                  for h in range(2):
                      nc.tensor.matmul(
                          out=ps[:, bank(h)], lhsT=w0k, rhs=xt[:, col(h)],
                          start=True, stop=False,
                      )
                  for h in range(2):
                      nc.tensor.matmul(
                          out=ps[:, bank(2 + h)], lhsT=w1k, rhs=xt[:, col(h)],
                          start=True, stop=False,
                      )
                  if t > 0:
                      for h in range(2):
                          nc.tensor.matmul(
                              out=ps[:, bank(h)], lhsT=Wm["e0"], rhs=xts[t - 1][:, col(h)],
                              start=False, stop=False,
                          )
                  if t < NT - 1:
                      for h in range(2):
                          nc.tensor.matmul(
                              out=ps[:, bank(2 + h)], lhsT=Wm["e1"], rhs=xts[t + 1][:, col(h)],
                              start=False, stop=False,
                          )
                  for q in range(4):
                      nc.tensor.matmul(
                          out=ps[:, bank(q)], lhsT=nilhs,
                          rhs=prhs[:, bank(q)],
                          start=False, stop=True, skip_group_check=True,
                      )

              slot = b * NT + t
              scr0 = scr_p.tile([128, 2 * W], bf16)
              nc.scalar.activation(
                  out=scr0[:], in_=ps[:], func=AF.Square,
                  accum_out=oimg_t[:, slot : slot + 1],
              )

          # ------------------- emission schedule -------------------
          if _parts == "stn":
              px, py, xs0, idx_t = emit_station_idx()
              g_dy = emit_station_gathers(idx_t)
              masks, rcnt = emit_station_masks(px, py, xs0)
              emit_stn_finalize(g_dy, masks, rcnt)
          else:
              emit_ploads(0)
              emit_xphase(0)
              if _parts == "both":
                  px, py, xs0, idx_t = emit_station_idx()
                  g_dy = emit_station_gathers(idx_t)
              emit_ploads(1)
              for t in range(NT):
                  emit_img_tile(0, t)
              emit_xphase(1)
              if _parts == "both":
                  masks, rcnt = emit_station_masks(px, py, xs0)
              emit_ploads(2)
              for t in range(NT):
                  emit_img_tile(1, t)
              emit_xphase(2)
              if _parts == "both":
                  emit_stn_finalize(g_dy, masks, rcnt)
              emit_ploads(3)
              for t in range(NT):
                  emit_img_tile(2, t)
              emit_xphase(3)
              for t in range(NT):
                  emit_img_tile(3, t)
              nc.sync.dma_start(out=o_img[:], in_=oimg_t[:])

    nc.compile()
    return nc


def _get_nc():
    if "nc" not in _CACHE:
        _CACHE["nc"] = _build_nc()
    return _CACHE["nc"]


def _in_maps(inputs):
    wconst, cconst, nif32 = _host_consts()
    preds = np.ascontiguousarray(np.asarray(inputs["predictions"], dtype=np.float32))
    targs = np.ascontiguousarray(np.asarray(inputs["targets"], dtype=np.float32))
    pos = np.ascontiguousarray(np.asarray(inputs["station_positions"], dtype=np.int32))
    runf = np.ascontiguousarray(np.asarray(inputs["station_runoffs"], dtype=np.float32))
    maps = []
    for c in range(N_CORES):
        sl = slice(c * B_LOC, (c + 1) * B_LOC)
        maps.append(
            {
                "predictions": np.ascontiguousarray(preds[sl]),
                "targets": np.ascontiguousarray(targs[sl]),
                "station_positions": np.ascontiguousarray(pos[sl]),
                "station_runoffs": np.ascontiguousarray(runf[sl]),
                "wconst": wconst,
                "cconst": cconst,
                "nif32": nif32,
            }
        )
    return maps


def _postprocess(results):
    img_sse = 0.0
    stn_sse = 0.0
    for r in results:
        img_sse += float(r["o_img"].astype(np.float64).sum())
        stn_sse += float(r["o_stn"].astype(np.float64).sum())
    img_loss = img_sse / float(B_TOT * H * W)
    stn_loss = stn_sse / float(B_TOT * S)
    total = 1.0 * img_loss + 0.5 * stn_loss
    return (
        np.float32(total),
        np.float32(img_loss),
        np.float32(stn_loss),
    )


def run(inputs, **run_kwargs):
    """Run the kernel; returns (BassKernelResults, (total, img, stn))."""
    from concourse.bass_utils import run_bass_kernel_spmd

    nc = _get_nc()
    res = run_bass_kernel_spmd(
        nc, _in_maps(inputs), core_ids=list(range(N_CORES)), **run_kwargs
    )
    return res, _postprocess(res.results)


def kernel(**inputs):
    _, out = run(inputs)
    return out


# revision 18
# speedup vs baseline: 1.1953x; 1.0737x over previous
"""nn_CombinedLoss Trainium2 kernel (v2).

Computes total/image/station losses for the CombinedLoss module, data-parallel
over the batch dim across 8 NeuronCores.

Per-core device pipeline (B_loc = 4 batches):
  Image loss  mean((P - bilinear_up2x(T))^2):
    - T row-tiles [128,512] -> fused x-upsample (scalar_tensor_tensor) into an
      interleaved bf16 row tile X' (scaled by 4/3 so the scale folds into the
      y-upsample band-matrix weights).
    - y-upsample + subtraction of P as PE matmuls with constant band matrices
      accumulating d = U - P in PSUM (even/odd fine-row quadrants). P enters
      the PE as raw float32 via a float32r bitcast (full rate at ramped
      p-state, exact precision) -- no bf16 cast pass on the Vector engine.
    - ScalarE Square activation with accum_out reduces each PSUM block to
      per-partition partial SSEs.
  Station loss  mean((clipped 3x3 box mean at station - runoff)^2):
    - per station, 3 row-segments of 4 consecutive pixels are gathered with one
      indirect DMA (element-offset table computed on device from positions).
    - masks/counts from positions; masked sum / count; squared diff reduced.
Host sums the per-core partials in f64.

Queue layout: sync issues the big P/T streaming loads; scalar issues the
constant loads (and runs the Square accumulation); gpsimd owns the station
gathers + finalize; vector builds X' and the station index/mask math.
"""

import os
import numpy as np
import ml_dtypes

N_CORES = 8
B_TOT, H, W = 32, 1024, 1024
TH, TW = 512, 512
S = 256
B_LOC = B_TOT // N_CORES  # 4
NT = TH // 128  # 4 target row-tiles per batch
N_IMG_SLOTS = 2 * B_LOC * NT  # 32 (one ACT accum slot per PSUM chunk)

_CACHE = {}


def _host_consts():
    z = lambda: np.zeros((128, 128), np.float32)
    w0, w1, e0, e1, ni = z(), z(), z(), z(), z()
    for m in range(128):
        w0[m, m] = 0.5625
        if m >= 1:
            w0[m - 1, m] = 0.1875
        w1[m, m] = 0.5625
        if m <= 126:
            w1[m + 1, m] = 0.1875
        ni[m, m] = -1.0
    w0f = w0.copy()
    w0f[0, 0] = 0.75
    w1l = w1.copy()
    w1l[127, 127] = 0.75
    e0[127, 0] = 0.1875
    e1[0, 127] = 0.1875
    wc = np.concatenate([w0, w0f, w1, w1l, e0, e1, ni], axis=1)
    wconst = np.ascontiguousarray(wc.astype(ml_dtypes.bfloat16))
    nif32 = np.ascontiguousarray(ni)

    cconst = np.zeros((128, 8), np.float32)
    p = np.arange(128)
    cconst[:, 0] = (p // 32).astype(np.float32) * float(H * W)  # batch offset
    cconst[:, 1:5] = np.arange(4, dtype=np.float32)[None, :]  # j window offsets
    return wconst, np.ascontiguousarray(cconst), nif32


def _build_nc():
    import concourse.bacc as bacc
    import concourse.bass as bass
    import concourse.mybir as mybir
    from concourse.tile import TileContext
    from contextlib import ExitStack

    f32 = mybir.dt.float32
    f32r = mybir.dt.float32r
    bf16 = mybir.dt.bfloat16
    i32 = mybir.dt.int32
    AL = mybir.AluOpType
    AF = mybir.ActivationFunctionType
    AX = mybir.AxisListType

    # --- build-time knobs (A/B testing) -----------------------------------
    KNI = os.environ.get("KNI", "f32r")  # f32r | bf16: dtype of P into PE
    KMMW = os.environ.get("KMMW", "narrow")  # narrow (512-col, ISA max) | wide
    _parts = os.environ.get("KPARTS", "both")

    nc = bacc.Bacc(
        "TRN2",
        target_bir_lowering=False,
        debug=False,
        enable_asserts=False,
        num_devices=N_CORES,
    )

    # predictions declared float32r (same bits as f32 host-side) so the PE can
    # consume P at full rate; non-PE readers bitcast back to f32.
    pdt = f32r if KNI == "f32r" else f32
    preds = nc.dram_tensor("predictions", [B_LOC, 1, H, W], pdt, kind="ExternalInput")
    targs = nc.dram_tensor("targets", [B_LOC, 1, TH, TW], f32, kind="ExternalInput")
    pos = nc.dram_tensor("station_positions", [B_LOC, S, 2], i32, kind="ExternalInput")
    runf = nc.dram_tensor("station_runoffs", [B_LOC, S], f32, kind="ExternalInput")
    wc = nc.dram_tensor("wconst", [128, 7 * 128], bf16, kind="ExternalInput")
    cc = nc.dram_tensor("cconst", [128, 8], f32, kind="ExternalInput")
    nif = nc.dram_tensor("nif32", [128, 128], pdt, kind="ExternalInput")
    o_img = nc.dram_tensor("o_img", [128, N_IMG_SLOTS], f32, kind="ExternalOutput")
    o_stn = nc.dram_tensor("o_stn", [128, 1], f32, kind="ExternalOutput")

    with TileContext(nc) as tc:
      with ExitStack() as ctx:
          const_p = ctx.enter_context(tc.tile_pool(name="const", bufs=1))
          stn_p = ctx.enter_context(tc.tile_pool(name="stn", bufs=1))
          tt_p = ctx.enter_context(tc.tile_pool(name="ttp", bufs=8))
          x_p = ctx.enter_context(tc.tile_pool(name="xp", bufs=NT + 2))
          pr_p = ctx.enter_context(tc.tile_pool(name="prp", bufs=8))
          pb_p = ctx.enter_context(tc.tile_pool(name="pbp", bufs=3))
          scr_p = ctx.enter_context(tc.tile_pool(name="scrp", bufs=3))
          # 2-bank (1024-col f32) PSUM chunks, 4 in flight: the Square drain of
          # chunk i overlaps the matmul groups of chunks i+1..i+3, so the PE
          # never waits on the Scalar engine (8 banks total).
          ps_p = ctx.enter_context(tc.tile_pool(name="psp", bufs=4, space="PSUM"))

          # ---- constant loads: scalar queue (its own DGE lane, early) ----
          wtile = const_p.tile([128, 7 * 128], bf16)
          nc.scalar.dma_start(out=wtile[:], in_=wc[:])
          names = ["w0", "w0f", "w1", "w1l", "e0", "e1", "ni"]
          Wm = {k: wtile[:, 128 * i : 128 * (i + 1)] for i, k in enumerate(names)}
          ctile = const_p.tile([128, 8], f32)
          nc.scalar.dma_start(out=ctile[:], in_=cc[:])
          niftile = const_p.tile([128, 128], pdt)
          nc.scalar.dma_start(out=niftile[:], in_=nif[:])
          boff = ctile[:, 0:1]
          joff = ctile[:, 1:5]

          # station inputs: gpsimd SWDGE lane
          if _parts in ("both", "stn"):
            pos_t = stn_p.tile([128, 16], i32)
            nc.gpsimd.dma_start(
                out=pos_t[:], in_=pos[:].rearrange("b (s2 s8) k -> (b s2) (s8 k)", s8=8)
            )
            run_t = stn_p.tile([128, 8], f32)
            nc.gpsimd.dma_start(
                out=run_t[:], in_=runf[:].rearrange("b (s2 s8) -> (b s2) s8", s8=8)
            )

          oimg_t = stn_p.tile([128, N_IMG_SLOTS], f32)
          ostn_t = stn_p.tile([128, 1], f32)

          third = 1.0 / 3.0
          xts_by_b = {}
          pfulls_by_b = {}

          def emit_ploads(b):
              pfulls = []
              for t in range(NT):
                  # T tile first (small; X' build gates the first matmuls)
                  ttile = tt_p.tile([128, TW], f32)
                  nc.sync.dma_start(
                      out=ttile[:], in_=targs[b, 0, 128 * t : 128 * (t + 1), :]
                  )
                  # one contiguous 1 MB load: partition p holds fine rows
                  # (256t+2p, 256t+2p+1) -> [even-row cols | odd-row cols]
                  pfull = pr_p.tile([128, 2 * W], pdt)
                  r0 = 256 * t
                  nc.sync.dma_start(
                      out=pfull[:],
                      in_=preds[b, 0, r0 : r0 + 256, :].rearrange(
                          "(p two) w -> p (two w)", two=2
                      ),
                  )
                  pfulls.append((ttile, pfull))
              pfulls_by_b[b] = pfulls

          def emit_xphase(b):
              xts = []
              for t in range(NT):
                  ttile = pfulls_by_b[b][t][0]
                  xt = x_p.tile([128, 2 * TW], bf16)
                  # even fine cols: X'[2i] = T[i] + T[i-1]/3   (i = 1..511)
                  nc.vector.scalar_tensor_tensor(
                      out=xt[:, 2 : 2 * TW : 2],
                      in0=ttile[:, 0 : TW - 1],
                      scalar=third,
                      in1=ttile[:, 1:TW],
                      op0=AL.mult,
                      op1=AL.add,
                  )
                  nc.vector.tensor_scalar(xt[:, 0:1], ttile[:, 0:1], 4.0 / 3.0, None, AL.mult)
                  # odd fine cols: X'[2i+1] = T[i] + T[i+1]/3  (i = 0..510)
                  nc.vector.scalar_tensor_tensor(
                      out=xt[:, 1 : 2 * TW - 1 : 2],
                      in0=ttile[:, 1:TW],
                      scalar=third,
                      in1=ttile[:, 0 : TW - 1],
                      op0=AL.mult,
                      op1=AL.add,
                  )
                  nc.vector.tensor_scalar(
                      xt[:, 2 * TW - 1 : 2 * TW], ttile[:, TW - 1 : TW], 4.0 / 3.0, None, AL.mult
                  )
                  xts.append(xt)
              xts_by_b[b] = xts

          def emit_station_idx():
            # vector-side index/mask math for the station gathers
            posf = stn_p.tile([128, 16], f32)
            nc.vector.tensor_copy(posf[:], pos_t[:])
            px = posf[:, 0:16:2]
            py = posf[:, 1:16:2]

            xs0 = stn_p.tile([128, 8], f32)
            nc.vector.tensor_scalar(xs0[:], px, -1.0, 0.0, AL.add, AL.max)
            nc.vector.tensor_scalar(xs0[:], xs0[:], float(W - 4), None, AL.min)

            # indices: fidx[p, s*3 + dyi] = clip(py+dy)*W + xs0  (+ batch offset)
            fidx = stn_p.tile([128, 24], f32)
            yc = stn_p.tile([128, 8], f32)
            for dyi, dy in enumerate((-1.0, 0.0, 1.0)):
                nc.vector.tensor_scalar(yc[:], py, dy, 0.0, AL.add, AL.max)
                nc.vector.tensor_scalar(yc[:], yc[:], float(H - 1), None, AL.min)
                nc.vector.scalar_tensor_tensor(
                    out=fidx[:, dyi:24:3],
                    in0=yc[:],
                    scalar=float(W),
                    in1=xs0[:],
                    op0=AL.mult,
                    op1=AL.add,
                )
            nc.vector.tensor_scalar(fidx[:], fidx[:], boff, None, AL.add)
            idx_t = stn_p.tile([128, 24], i32)
            nc.vector.tensor_copy(idx_t[:], fidx[:])
            return px, py, xs0, idx_t

          def emit_station_gathers(idx_t):
            # HW indirect DMA honors ONE index per partition per instruction
            # (verified: extra free-dim indices are ignored; the transfer is
            # out-free-size contiguous elements from the first index). So:
            # 24 gathers, one per (station-slot s, dy) pair. gpsimd-only.
            # Three per-dy destination tiles, gathers round-robin across them:
            # consecutive gathers hit different tiles so the WAW completion
            # chain (desc-gen + wire + sem ~2us) never stalls the queue.
            g_dy = [stn_p.tile([128, 32], f32, name=f"g_dy{i}") for i in range(3)]
            pred_flat = preds[:].bitcast(f32).rearrange("b c h w -> (b c h) w")
            for s in range(8):
                for dyi in range(3):
                    nc.gpsimd.indirect_dma_start(
                        out=g_dy[dyi][:, 4 * s : 4 * s + 4],
                        out_offset=None,
                        in_=pred_flat,
                        in_offset=bass.IndirectOffsetOnAxis(
                            ap=idx_t[:, 3 * s + dyi : 3 * s + dyi + 1], axis=1
                        ),
                    )
            return g_dy

          def emit_station_masks(px, py, xs0):
            tx = stn_p.tile([128, 8], f32)
            nc.vector.tensor_tensor(tx[:], xs0[:], px, AL.subtract)
            # wx[:, 4s+j] = |xs0_s + j - px_s| <= 1  -- x-window mask, (s,j)
            wx = stn_p.tile([128, 32], f32)
            nc.vector.tensor_tensor(
                wx[:].rearrange("p (s j) -> p s j", j=4),
                tx[:].unsqueeze(2).broadcast_to([128, 8, 4]),
                joff.unsqueeze(1).broadcast_to([128, 8, 4]),
                AL.add,
            )
            nc.vector.tensor_tensor(wx[:], wx[:], wx[:], AL.mult)
            nc.vector.tensor_scalar(wx[:], wx[:], 1.5, None, AL.is_le)

            # per-dy row-validity (dy=0 is always valid: py in [0, H))
            vy_m1 = stn_p.tile([128, 8], f32)
            nc.vector.tensor_scalar(vy_m1[:], py, 1.0, None, AL.is_ge)
            vy_p1 = stn_p.tile([128, 8], f32)
            nc.vector.tensor_scalar(vy_p1[:], py, float(H - 2), None, AL.is_le)

            # masked per-dy (s,j) masks: mask_dy = wx * vy_dy
            mask_m1 = stn_p.tile([128, 32], f32)
            nc.vector.tensor_tensor(
                mask_m1[:].rearrange("p (s j) -> p s j", j=4),
                wx[:].rearrange("p (s j) -> p s j", j=4),
                vy_m1[:].unsqueeze(2).broadcast_to([128, 8, 4]),
                AL.mult,
            )
            mask_p1 = stn_p.tile([128, 32], f32)
            nc.vector.tensor_tensor(
                mask_p1[:].rearrange("p (s j) -> p s j", j=4),
                wx[:].rearrange("p (s j) -> p s j", j=4),
                vy_p1[:].unsqueeze(2).broadcast_to([128, 8, 4]),
                AL.mult,
            )

            cy = stn_p.tile([128, 8], f32)
            nc.vector.tensor_tensor(cy[:], vy_m1[:], vy_p1[:], AL.add)
            nc.vector.tensor_scalar(cy[:], cy[:], 1.0, None, AL.add)
            cx = stn_p.tile([128, 8], f32)
            nc.vector.tensor_reduce(
                cx[:], wx[:].rearrange("p (s j) -> p s j", j=4), AX.X, AL.add
            )
            cnt = stn_p.tile([128, 8], f32)
            nc.vector.tensor_tensor(cnt[:], cy[:], cx[:], AL.mult)
            rcnt = stn_p.tile([128, 8], f32)
            nc.vector.reciprocal(rcnt[:], cnt[:])
            return (mask_m1, wx, mask_p1), rcnt

          def emit_stn_finalize(g_dy, masks, rcnt):
            # all on GPSIMD: keeps gather-dependent work off the (busy, in-order)
            # Vector queue so a scheduler mis-ordering can't head-of-line block it
            acc = stn_p.tile([128, 32], f32)
            gm1 = stn_p.tile([128, 32], f32)
            nc.gpsimd.tensor_tensor(acc[:], g_dy[0][:], masks[0][:], AL.mult)
            nc.gpsimd.tensor_tensor(gm1[:], g_dy[1][:], masks[1][:], AL.mult)
            nc.gpsimd.tensor_tensor(acc[:], acc[:], gm1[:], AL.add)
            nc.gpsimd.tensor_tensor(gm1[:], g_dy[2][:], masks[2][:], AL.mult)
            nc.gpsimd.tensor_tensor(acc[:], acc[:], gm1[:], AL.add)
            # j-tree: (s,4) -> (s,2) -> (s,1)
            accv = acc[:].rearrange("p (s j) -> p s j", j=4)
            h2 = stn_p.tile([128, 16], f32)
            h2v = h2[:].rearrange("p (s j) -> p s j", j=2)
            nc.gpsimd.tensor_tensor(h2v, accv[:, :, 0:2], accv[:, :, 2:4], AL.add)
            bsum = stn_p.tile([128, 8], f32)
            nc.gpsimd.tensor_tensor(bsum[:], h2v[:, :, 0], h2v[:, :, 1], AL.add)
            d_t = stn_p.tile([128, 8], f32)
            nc.gpsimd.tensor_tensor(d_t[:], bsum[:], rcnt[:], AL.mult)
            nc.gpsimd.tensor_tensor(d_t[:], d_t[:], run_t[:], AL.subtract)
            scr8 = stn_p.tile([128, 8], f32)
            nc.gpsimd.tensor_tensor(scr8[:], d_t[:], d_t[:], AL.mult)
            s4 = stn_p.tile([128, 4], f32)
            nc.gpsimd.tensor_tensor(s4[:], scr8[:, 0:4], scr8[:, 4:8], AL.add)
            s2 = stn_p.tile([128, 2], f32)
            nc.gpsimd.tensor_tensor(s2[:], s4[:, 0:2], s4[:, 2:4], AL.add)
            nc.gpsimd.tensor_tensor(ostn_t[:], s2[:, 0:1], s2[:, 1:2], AL.add)
            # o_stn goes out on gpsimd's own SWDGE lane: it must NOT sit in the
            # sync queue mid-stream, where it would head-of-line block the
            # remaining P/T load issues behind the whole station dep chain.
            nc.gpsimd.dma_start(out=o_stn[:], in_=ostn_t[:])

          def emit_img_chunk(b, t, q):
              # q=0: even fine rows (w0 band, e0 halo, P even-row cols)
              # q=1: odd fine rows (w1 band, e1 halo, P odd-row cols)
              xts = xts_by_b[b]
              pfull = pfulls_by_b[b][t][1]
              xt = xts[t]
              ps = ps_p.tile([128, W], f32, space="PSUM", name="ps")
              if q == 0:
                  wk = Wm["w0f"] if t == 0 else Wm["w0"]
                  ek, xe = (Wm["e0"], xts[t - 1]) if t > 0 else (None, None)
              else:
                  wk = Wm["w1l"] if t == NT - 1 else Wm["w1"]
                  ek, xe = (Wm["e1"], xts[t + 1]) if t < NT - 1 else (None, None)

              if KNI == "f32r":
                  prhs = pfull[:, 1024 * q : 1024 * (q + 1)]
                  nilhs = niftile[:]
              else:
                  pbf = pb_p.tile([128, W], bf16)
                  nc.vector.tensor_copy(pbf[:], pfull[:, 1024 * q : 1024 * (q + 1)])
                  prhs = pbf[:]
                  nilhs = Wm["ni"]

              bank = lambda h: slice(512 * h, 512 * (h + 1))
              for h in range(2):
                  nc.tensor.matmul(
                      out=ps[:, bank(h)], lhsT=wk, rhs=xt[:, bank(h)],
                      start=True, stop=False,
                  )
              if ek is not None:
                  for h in range(2):
                      nc.tensor.matmul(
                          out=ps[:, bank(h)], lhsT=ek, rhs=xe[:, bank(h)],
                          start=False, stop=False,
                      )
              for h in range(2):
                  nc.tensor.matmul(
                      out=ps[:, bank(h)], lhsT=nilhs, rhs=prhs[:, bank(h)],
                      start=False, stop=True, skip_group_check=True,
                  )

              slot = 2 * (b * NT + t) + q
              scr0 = scr_p.tile([128, W], bf16)
              nc.scalar.activation(
                  out=scr0[:], in_=ps[:], func=AF.Square,
                  accum_out=oimg_t[:, slot : slot + 1],
              )

          def emit_img_tile(b, t):
              emit_img_chunk(b, t, 0)
              emit_img_chunk(b, t, 1)

          # ------------------- emission schedule -------------------
          if _parts == "stn":
              px, py, xs0, idx_t = emit_station_idx()
              g_dy = emit_station_gathers(idx_t)
              masks, rcnt = emit_station_masks(px, py, xs0)
              emit_stn_finalize(g_dy, masks, rcnt)
          else:
              emit_ploads(0)
              emit_xphase(0)
              if _parts == "both":
                  px, py, xs0, idx_t = emit_station_idx()
                  g_dy = emit_station_gathers(idx_t)
              emit_ploads(1)
              for t in range(NT):
                  emit_img_tile(0, t)
              emit_xphase(1)
              if _parts == "both":
                  masks, rcnt = emit_station_masks(px, py, xs0)
              emit_ploads(2)
              for t in range(NT):
                  emit_img_tile(1, t)
              emit_xphase(2)
              if _parts == "both":
                  emit_stn_finalize(g_dy, masks, rcnt)
              emit_ploads(3)
              for t in range(NT):
                  emit_img_tile(2, t)
              emit_xphase(3)
              for t in range(NT):
                  emit_img_tile(3, t)
              nc.sync.dma_start(out=o_img[:], in_=oimg_t[:])

    nc.compile()
    return nc


def _get_nc():
    if "nc" not in _CACHE:
        _CACHE["nc"] = _build_nc()
    return _CACHE["nc"]


def _in_maps(inputs):
    wconst, cconst, nif32 = _host_consts()
    preds = np.ascontiguousarray(np.asarray(inputs["predictions"], dtype=np.float32))
    targs = np.ascontiguousarray(np.asarray(inputs["targets"], dtype=np.float32))
    pos = np.ascontiguousarray(np.asarray(inputs["station_positions"], dtype=np.int32))
    runf = np.ascontiguousarray(np.asarray(inputs["station_runoffs"], dtype=np.float32))
    maps = []
    for c in range(N_CORES):
        sl = slice(c * B_LOC, (c + 1) * B_LOC)
        maps.append(
            {
                "predictions": np.ascontiguousarray(preds[sl]),
                "targets": np.ascontiguousarray(targs[sl]),
                "station_positions": np.ascontiguousarray(pos[sl]),
                "station_runoffs": np.ascontiguousarray(runf[sl]),
                "wconst": wconst,
                "cconst": cconst,
                "nif32": nif32,
            }
        )
    return maps


def _postprocess(results):
    img_sse = 0.0
    stn_sse = 0.0
    for r in results:
        img_sse += float(r["o_img"].astype(np.float64).sum())
        stn_sse += float(r["o_stn"].astype(np.float64).sum())
    img_loss = img_sse / float(B_TOT * H * W)
    stn_loss = stn_sse / float(B_TOT * S)
    total = 1.0 * img_loss + 0.5 * stn_loss
    return (
        np.float32(total),
        np.float32(img_loss),
        np.float32(stn_loss),
    )


def run(inputs, **run_kwargs):
    """Run the kernel; returns (BassKernelResults, (total, img, stn))."""
    from concourse.bass_utils import run_bass_kernel_spmd

    nc = _get_nc()
    res = run_bass_kernel_spmd(
        nc, _in_maps(inputs), core_ids=list(range(N_CORES)), **run_kwargs
    )
    return res, _postprocess(res.results)


def kernel(**inputs):
    _, out = run(inputs)
    return out
